# revision 69
# baseline (speedup 1.0000x reference)
"""Trainium2 Bass kernel for nn_Attention (B=4, S=2048, D=2048, H=16, KV=4, HD=128).

Sharding (8 cores): data-parallel over batch (4) x tensor-parallel over
KV-head-group halves (2). Core c handles batch b=c//2 and q-heads
[8*(c%2), 8*(c%2)+8) == kv groups {2*(c%2), 2*(c%2)+1}. Each core produces a
partial output (its heads' contribution through wo); the host sums the two
partials per batch.

Projections (x@wq, x@wkv) run in bf16 in a single merged sweep over x
(shared stationary x tiles, one x DMA; weights stream in behind the first
x tile). RoPE on DVE, then per-head PE transposes produce qT/kT in bf16;
V is stored bf16. Attention is computed transposed (scoresT[k,q]) in bf16
with fp32 PSUM accumulation: per (group) all 4 r's score blocks+exp issue
first (building ACT backlog), then the AV matmuls. Softmax denominators
come from tiny probs-stationary matmuls (out [128q,1] per 128-q-chunk,
~free on PE since LD_WEIGHTS shadows the 1-column stream), then DVE
reciprocal -> four skinny [128,1]->[1,128] PE transposes -> one rank-1
broadcast matmul back to [128,512], all time-sharing one PSUM bank
(matmul start=True zeroes the whole bank, so only each tile's first write
sets it); normalization is fused into the PSUM->SBUF move of the AV
result (DVE multiply). wo stays resident in SBUF (bf16) and the output
projection of superblock qsb-1 is interleaved into the AV phase of qsb so
PE keeps streaming while ACT drains the exp backlog; qsb0 issues both
groups' scores up front (A-A-B-B) since it has no out-proj filler.
The causal mask is folded into the scores matmul via an identity-
stationary accumulation (sc += I @ msk, mask clamped to finite bf16) so
exp never waits on DVE. TimelineSim: ~456us/core vs the 584us baseline;
rel err ~4.7e-3.
"""
import numpy as np

B, S, D = 4, 2048, 2048
H, KV, HD = 16, 4, 128
NREP = H // KV
SCALE = float(HD) ** -0.5

SB = S // 128          # 16 s-blocks
KT = D // 128          # 16 contraction tiles for projections
QSB = S // 512         # 4 q-superblocks
HPC = 8                # q heads per core
GPC = 2                # kv groups per core

_compiled = {}


def _build(causal: bool):
    import concourse.bass as bass  # noqa: F401
    import concourse.tile as tile
    from concourse import bacc, mybir
    from concourse.masks import make_identity

    f32 = mybir.dt.float32
    f32r = mybir.dt.float32r
    bf16 = mybir.dt.bfloat16
    AF = mybir.ActivationFunctionType
    ALU = mybir.AluOpType

    nc = bacc.Bacc("TRN2")

    xT = nc.dram_tensor("xT", [D, S], bf16, kind="ExternalInput")
    wqT = nc.dram_tensor("wqT", [D, HPC * HD], bf16, kind="ExternalInput")
    wkvT = nc.dram_tensor("wkvT", [D, 2 * GPC * HD], bf16, kind="ExternalInput")
    woT = nc.dram_tensor("woT", [HPC * HD, D], bf16, kind="ExternalInput")
    cosS = nc.dram_tensor("cosS", [128, SB, 64], f32, kind="ExternalInput")
    sinS = nc.dram_tensor("sinS", [128, SB, 64], f32, kind="ExternalInput")
    mtile = nc.dram_tensor("mtile", [128, 128], bf16, kind="ExternalInput")
    onest = nc.dram_tensor("onest", [128, 128], bf16, kind="ExternalInput")
    outT = nc.dram_tensor("outT", [D, S], bf16, kind="ExternalOutput")

    xT3 = xT.rearrange("(kt p) s -> p kt s", p=128)
    wqT3 = wqT.rearrange("(kt p) e -> p kt e", p=128)
    wkvT3 = wkvT.rearrange("(kt p) e -> p kt e", p=128)
    woT3 = woT.rearrange("(h p) d -> p h d", p=128)

    with tile.TileContext(nc) as tc:
        with tc.tile_pool(name="persist", bufs=1) as persist:
            qT = [persist.tile([128, S], bf16, tag=f"qT{h}", name=f"qT{h}") for h in range(HPC)]
            kT = [persist.tile([128, S], bf16, tag=f"kTg{g}", name=f"kTg{g}") for g in range(GPC)]
            vsb = [persist.tile([128, SB, 128], bf16, tag=f"v{g}", name=f"v{g}") for g in range(GPC)]
            msk = persist.tile([128, 128], bf16, tag="msk")
            ones = persist.tile([128, 128], bf16, tag="ones")
            identp = persist.tile([128, 128], bf16, tag="identp")

            # ---------- Stage 1: merged q+kv projections + RoPE ----------
            s1ctx = tc.tile_pool(name="s1const", bufs=1)
            s1const = s1ctx.__enter__()
            ident_f = s1const.tile([128, 128], f32, tag="identf")
            make_identity(nc, ident_f)
            ident = identp
            nc.vector.tensor_copy(out=ident, in_=ident_f)
            cos_t = s1const.tile([128, SB, 64], f32, tag="cos")
            sin_t = s1const.tile([128, SB, 64], f32, tag="sin")
            wq_t = s1const.tile([128, KT, HPC * HD], bf16, tag="wq")
            wkv_t = s1const.tile([128, KT, 2 * GPC * HD], bf16, tag="wkv")

            with tc.tile_pool(name="xs1", bufs=2) as xpool, \
                 tc.tile_pool(name="rs1", bufs=2) as rpool, \
                 tc.tile_pool(name="pq1", bufs=2, space="PSUM") as pqp, \
                 tc.tile_pool(name="pk1", bufs=2, space="PSUM") as pkp, \
                 tc.tile_pool(name="pt1", bufs=2, space="PSUM") as ptp:
                for sb in range(SB):
                    xs = xpool.tile([128, KT, 128], bf16, tag="xs")
                    xeng = nc.sync if sb == 0 else nc.scalar
                    xeng.dma_start(
                        out=xs[:, 0:8, :],
                        in_=xT3[:, 0:8, sb * 128:(sb + 1) * 128])
                    xeng.dma_start(
                        out=xs[:, 8:16, :],
                        in_=xT3[:, 8:16, sb * 128:(sb + 1) * 128])
                    if sb == 0:
                        for kt4 in range(0, KT, 2):
                            nc.sync.dma_start(
                                out=wkv_t[:, kt4:kt4 + 2, :],
                                in_=wkvT3[:, kt4:kt4 + 2, :])
                            nc.sync.dma_start(
                                out=wq_t[:, kt4:kt4 + 2, :],
                                in_=wqT3[:, kt4:kt4 + 2, :])
                            if kt4 == 2:
                                nc.sync.dma_start(out=cos_t, in_=cosS[:, :, :])
                                nc.sync.dma_start(out=sin_t, in_=sinS[:, :, :])
                        # mask/ones aren't needed until attention
                        nc.scalar.dma_start(out=msk, in_=mtile[:, :])
                        nc.scalar.dma_start(out=ones, in_=onest[:, :])
                    ps_q = pqp.tile([128, HPC * HD], f32, tag="psq")
                    ps_k = pkp.tile([128, 2 * GPC * HD], f32, tag="psk")
                    kts = list(range(KT))
                    for i, kt in enumerate(kts):
                        nc.tensor.matmul(
                            ps_k, xs[:, kt, :], wkv_t[:, kt, :],
                            start=(i == 0), stop=(i == KT - 1))
                        for n0 in (0, 512):
                            nc.tensor.matmul(
                                ps_q[:, n0:n0 + 512], xs[:, kt, :],
                                wq_t[:, kt, n0:n0 + 512],
                                start=(i == 0), stop=(i == KT - 1))

                    # V copies (no RoPE) — ACT engine
                    ps_k3 = ps_k.rearrange("p (h d) -> p h d", d=128)
                    for g in range(GPC):
                        nc.scalar.copy(
                            out=vsb[g][:, sb, :], in_=ps_k3[:, GPC + g, :])

                    # RoPE on q (8 heads) and k (2 heads)
                    def rope(ps3, h0, nr, tagsfx, rp=None):
                        if rp is None:
                            rp = rpool.tile([128, HPC, 128], bf16,
                                            tag="rp" + tagsfx, name="rp")
                        ev = ps3[:, h0:h0 + nr, 0:128:2]
                        od = ps3[:, h0:h0 + nr, 1:128:2]
                        cb = cos_t[:, None, sb, :].broadcast_to([128, nr, 64])
                        sn = sin_t[:, None, sb, :].broadcast_to([128, nr, 64])
                        t1 = rpool.tile([128, HPC, 64], f32, tag="t1" + tagsfx)
                        t2 = rpool.tile([128, HPC, 64], f32, tag="t2" + tagsfx)
                        hs = slice(h0, h0 + nr)
                        nc.vector.tensor_tensor(
                            out=t1[:, 0:nr, :], in0=ev, in1=cb, op=ALU.mult)
                        nc.vector.tensor_tensor(
                            out=t2[:, 0:nr, :], in0=od, in1=sn, op=ALU.mult)
                        nc.vector.tensor_tensor(
                            out=rp[:, hs, 0:64], in0=t1[:, 0:nr, :],
                            in1=t2[:, 0:nr, :], op=ALU.subtract)
                        nc.vector.tensor_tensor(
                            out=t1[:, 0:nr, :], in0=ev, in1=sn, op=ALU.mult)
                        nc.vector.tensor_tensor(
                            out=t2[:, 0:nr, :], in0=od, in1=cb, op=ALU.mult)
                        nc.vector.tensor_tensor(
                            out=rp[:, hs, 64:128], in0=t1[:, 0:nr, :],
                            in1=t2[:, 0:nr, :], op=ALU.add)
                        return rp

                    def kt_transp(g, rpk):
                        pt = ptp.tile([128, 128], bf16, tag="pt")
                        nc.tensor.transpose(pt, rpk[:, g, :], ident)
                        nc.scalar.copy(
                            out=kT[g][:, sb * 128:(sb + 1) * 128], in_=pt)

                    def qt_transp(h, rpq):
                        pt = ptp.tile([128, 128], bf16, tag="pt")
                        nc.tensor.transpose(pt, rpq[:, h, :], ident)
                        nc.scalar.copy(
                            out=qT[h][:, sb * 128:(sb + 1) * 128], in_=pt)

                    ps_q3 = ps_q.rearrange("p (h d) -> p h d", d=128)
                    rpk = rope(ps_k3, 0, GPC, "k")
                    if sb < SB - 1:
                        rpq = rope(ps_q3, 0, HPC, "q")
                        for g in range(GPC):
                            kt_transp(g, rpk)
                        for h in range(HPC):
                            qt_transp(h, rpq)
                    else:
                        # last s-block: halve the rope->transpose latency so
                        # stage 2 (queued behind the transposes) starts early
                        for g in range(GPC):
                            kt_transp(g, rpk)
                        rpq = rope(ps_q3, 0, 4, "q")
                        for h in range(4):
                            qt_transp(h, rpq)
                        rpq = rope(ps_q3, 4, 4, "q", rp=rpq)
                        for h in range(4, HPC):
                            qt_transp(h, rpq)
            s1ctx.__exit__(None, None, None)

            # ---------- Stage 2: attention + interleaved out-projection ----
            with tc.tile_pool(name="s2", bufs=1) as s2pool, \
                 tc.tile_pool(name="pr2", bufs=5) as prpool, \
                 tc.tile_pool(name="att2", bufs=2) as attpool, \
                 tc.tile_pool(name="dn2", bufs=2) as dnpool, \
                 tc.tile_pool(name="o2", bufs=3) as opool, \
                 tc.tile_pool(name="pav", bufs=2, space="PSUM") as pavp, \
                 tc.tile_pool(name="pdn", bufs=1, space="PSUM") as pdnp, \
                 tc.tile_pool(name="pou", bufs=2, space="PSUM") as poup, \
                 tc.tile_pool(name="psc", bufs=3, space="PSUM") as pscp:
                wos = s2pool.tile([128, HPC, D], bf16, tag="wos")
                for h2 in range(0, HPC, 2):
                    nc.scalar.dma_start(
                        out=wos[:, h2:h2 + 2, :], in_=woT3[:, h2:h2 + 2, :])

                att_prev = None

                def op_block(m, qsb_prev, att_p, split=False):
                    po = poup.tile([128, 512], f32, tag="po")
                    for e in range(HPC):
                        nc.tensor.matmul(
                            po, wos[:, e, m * 128:(m + 1) * 128],
                            att_p[:, e, :], start=(e == 0), stop=(e == HPC - 1))
                    ot = opool.tile([128, 512], bf16, tag="ot")
                    od = outT[m * 128:(m + 1) * 128,
                              qsb_prev * 512:(qsb_prev + 1) * 512]
                    if split:
                        # drain the final block via both copy engines
                        nc.vector.tensor_copy(out=ot[:, 0:256], in_=po[:, 0:256])
                        nc.sync.dma_start(out=od[:, 0:256], in_=ot[:, 0:256])
                        nc.scalar.copy(out=ot[:, 256:512], in_=po[:, 256:512])
                        nc.scalar.dma_start(out=od[:, 256:512],
                                            in_=ot[:, 256:512])
                    else:
                        nc.vector.tensor_copy(out=ot, in_=po)
                        nc.sync.dma_start(out=od, in_=ot)

                def phase_a(qsb, g, maxkt, q0g, tag):
                    prb = []
                    for r in range(NREP):
                        h = g * NREP + r
                        shp = [128, maxkt, 512] if tag == "probs0" \
                            else [128, SB, 512]
                        probs = prpool.tile(shp, bf16, tag=tag)
                        prb.append(probs)
                        for t in range(maxkt):
                            ql = max(0, t * 128 - q0g) if causal else 0
                            sc = pscp.tile([128, 512], f32, tag="sc")
                            nc.tensor.matmul(
                                sc[:, ql:512],
                                kT[g][:, t * 128:(t + 1) * 128],
                                qT[h][:, q0g + ql:q0g + 512],
                                start=True,
                                stop=not (causal and t * 128 >= q0g),
                                skip_group_check=True)
                            is_diag = causal and t * 128 >= q0g
                            if is_diag:
                                # diag block: fold the 0/-inf mask in via an
                                # identity-stationary matmul accumulation
                                # (sc += I @ msk) so exp never waits on DVE;
                                # exp(SCALE*(sc+msk)) underflows to 0
                                # identically for the 0/-inf mask
                                nc.tensor.matmul(
                                    sc[:, ql:ql + 128], identp, msk,
                                    start=False, stop=True,
                                    skip_group_check=True)
                            nc.scalar.activation(
                                out=probs[:, t, ql:512],
                                in_=sc[:, ql:512], func=AF.Exp,
                                scale=SCALE)
                    return prb

                def phase_b(qsb, g, maxkt, q0g, prb, att, it, att_p,
                            pending):
                    # pending: deferred chain-tail (transposes/broadcast/
                    # normalize) of the previous r, emitted one iteration
                    # late so its DVE round-trips hide under AV/out-proj work
                    def chain_tail(av, rrq, h):
                        def emit():
                            # reorient: 4 skinny transposes land each chunk's
                            # recip row at partition 0 of one PSUM row
                            ptc = pdnp.tile([1, 512], bf16, tag="dn")
                            for c in range(4):
                                nc.tensor.matmul(
                                    ptc[0:1, c * 128:(c + 1) * 128],
                                    rrq[:, c:c + 1], identp,
                                    is_transpose=True,
                                    start=(c == 0), stop=(c == 3),
                                    skip_group_check=True)
                            rr = dnpool.tile([1, 512], bf16, tag="rr")
                            nc.vector.tensor_copy(out=rr, in_=ptc)
                            # broadcast recip row to [128, 512]
                            rsb = pdnp.tile([128, 512], f32, tag="dn")
                            nc.tensor.matmul(
                                rsb, ones[0:1, :], rr,
                                start=True, stop=True,
                                skip_group_check=True)
                            rsbs = dnpool.tile([128, 512], bf16, tag="rsbs")
                            nc.vector.tensor_copy(out=rsbs, in_=rsb)
                            # normalize: att = av * rsb (PSUM->SBUF fused)
                            nc.vector.tensor_tensor(
                                out=att[:, h, :], in0=av, in1=rsbs,
                                op=ALU.mult)
                        return emit

                    for r in range(NREP):
                        h = g * NREP + r
                        probs = prb[r]
                        av = pavp.tile([128, 512], f32, tag="av")
                        for t in range(maxkt):
                            ql = max(0, t * 128 - q0g) if causal else 0
                            nc.tensor.matmul(
                                av[:, ql:512], vsb[g][:, t, :],
                                probs[:, t, ql:512],
                                start=(t == 0), stop=(t == maxkt - 1),
                                skip_group_check=True)
                        # denominators: probs-stationary tiny matmuls,
                        # accumulated per 128-q-chunk in PSUM
                        # NOTE: matmul start=True zeroes the whole PSUM
                        # bank, so only the very first write may set it
                        den = pdnp.tile([128, 4], f32, tag="dn")
                        for t in range(maxkt):
                            ql = max(0, t * 128 - q0g) if causal else 0
                            for c in range(ql // 128, 4):
                                tstop = (4 * qsb + c) if causal else (maxkt - 1)
                                nc.tensor.matmul(
                                    den[:, c:c + 1],
                                    probs[:, t, c * 128:(c + 1) * 128],
                                    ones[:, 0:1],
                                    start=(t == 0 and c == 0),
                                    stop=(t == tstop),
                                    skip_group_check=True)
                        rrq = dnpool.tile([128, 4], bf16, tag="rrq")
                        with nc.allow_low_precision(reason="softmax recip"):
                            nc.vector.reciprocal(out=rrq, in_=den)
                        # out-projection fill for the previous superblock
                        # (also gives the reciprocal time to land before the
                        # chain tail needs it)
                        if att_p is not None:
                            for m in (2 * it, 2 * it + 1):
                                op_block(m, qsb - 1, att_p)
                        chain_tail(av, rrq, h)()
                        it += 1
                    return it, pending

                pending = None
                for qsb in range(QSB):
                    att = attpool.tile([128, HPC, 512], bf16, tag="att")
                    maxkt = (qsb + 1) * 4 if causal else SB
                    q0g = qsb * 512
                    it = 0  # (g, r) iteration counter within this qsb
                    if qsb == 0 and causal:
                        # A A B B: both groups' scores issue up front so the
                        # exp backlog drains while PE streams the second
                        # group's scores (no out-proj filler exists yet)
                        prbs = [phase_a(qsb, g, maxkt, q0g,
                                        "probs" if g == 0 else "probs0")
                                for g in range(GPC)]
                        for g in range(GPC):
                            it, pending = phase_b(qsb, g, maxkt, q0g,
                                                  prbs[g], att, it,
                                                  att_prev, pending)
                    else:
                        for g in range(GPC):
                            prb = phase_a(qsb, g, maxkt, q0g, "probs")
                            it, pending = phase_b(qsb, g, maxkt, q0g, prb,
                                                  att, it, att_prev, pending)
                    att_prev = att
                if pending is not None:
                    pending()
                # tail: out-projection of the last superblock
                for m in range(KT):
                    op_block(m, QSB - 1, att_prev)

    nc.compile()
    return nc


def _get_nc(causal: bool):
    if causal not in _compiled:
        _compiled[causal] = _build(causal)
    return _compiled[causal]


def kernel(x, freqs_cis, mask, wq, wk, wv, wo):
    import ml_dtypes
    from concourse.bass_utils import run_bass_kernel_spmd

    x = np.asarray(x, dtype=np.float32)
    freqs_cis = np.asarray(freqs_cis, dtype=np.float32)
    mask = np.asarray(mask, dtype=np.float32)
    wq = np.asarray(wq, dtype=np.float32)
    wk = np.asarray(wk, dtype=np.float32)
    wv = np.asarray(wv, dtype=np.float32)
    wo = np.asarray(wo, dtype=np.float32)

    tri = np.tril(np.ones((S, S), dtype=bool))
    causal = bool((mask[tri] == 0.0).all() and (mask[~tri] < -1e30).all())
    if not causal and not (mask == 0.0).all():
        return _numpy_ref(x, freqs_cis, mask, wq, wk, wv, wo)

    nc = _get_nc(causal)

    cos = freqs_cis[:, :, 0]
    sin = freqs_cis[:, :, 1]
    cosS = np.ascontiguousarray(cos.reshape(SB, 128, 64).transpose(1, 0, 2))
    sinS = np.ascontiguousarray(sin.reshape(SB, 128, 64).transpose(1, 0, 2))
    # clamp to a finite bf16: -float32-max would round to -inf, and the
    # identity-matmul mask fold would then compute 0 * -inf = NaN
    mtile = np.maximum(
        (np.ascontiguousarray(mask[0:128, 0:128].T) if causal
         else np.zeros((128, 128), dtype=np.float32)), -3.0e38
    ).astype(ml_dtypes.bfloat16)
    onest = np.ones((128, 128), dtype=ml_dtypes.bfloat16)

    in_maps = []
    for c in range(8):
        b, i = c // 2, c % 2
        in_maps.append({
            "xT": np.ascontiguousarray(x[b].T).astype(ml_dtypes.bfloat16),
            "wqT": np.ascontiguousarray(
                wq[1024 * i:1024 * (i + 1), :].T).astype(ml_dtypes.bfloat16),
            "wkvT": np.ascontiguousarray(np.concatenate(
                [wk[256 * i:256 * (i + 1), :].T,
                 wv[256 * i:256 * (i + 1), :].T],
                axis=1)).astype(ml_dtypes.bfloat16),
            "woT": np.ascontiguousarray(
                wo[:, 1024 * i:1024 * (i + 1)].T).astype(ml_dtypes.bfloat16),
            "cosS": cosS, "sinS": sinS, "mtile": mtile, "onest": onest,
        })

    out = None
    for _attempt in range(2):
        res = run_bass_kernel_spmd(nc, in_maps, core_ids=list(range(8)))
        out = np.empty((B, S, D), dtype=np.float32)
        for b in range(B):
            o0 = res.results[2 * b]["outT"].astype(np.float32)
            o1 = res.results[2 * b + 1]["outT"].astype(np.float32)
            out[b] = o0.T + o1.T
        if np.isfinite(out).all():
            break
        # transient device-warmup glitch observed once on the axon path:
        # retry a single time rather than returning garbage
    return out


def _numpy_ref(x, freqs_cis, mask, wq, wk, wv, wo):
    xq = (x @ wq.T).reshape(B, S, H, HD)
    xk = (x @ wk.T).reshape(B, S, KV, HD)
    xv = (x @ wv.T).reshape(B, S, KV, HD)

    def rope(xh):
        x2 = xh.reshape(*xh.shape[:-1], HD // 2, 2)
        fc = freqs_cis[None, :, None, :, :]
        real = x2[..., 0] * fc[..., 0] - x2[..., 1] * fc[..., 1]
        imag = x2[..., 0] * fc[..., 1] + x2[..., 1] * fc[..., 0]
        return np.concatenate([real, imag], axis=-1)

    xq, xk = rope(xq), rope(xk)
    q = xq.reshape(B, S, KV, NREP, HD)
    sc = np.einsum('bqgrd,bkgd->bgrqk', q, xk) * SCALE + mask[None, None, None]
    sc = sc - sc.max(axis=-1, keepdims=True)
    p = np.exp(sc)
    p /= p.sum(axis=-1, keepdims=True)
    o = np.einsum('bgrqk,bkgd->bqgrd', p, xv).reshape(B, S, H * HD)
    return (o @ wo.T).astype(np.float32)


# revision 71
# speedup vs baseline: 1.0004x; 1.0004x over previous
"""Trainium2 Bass kernel for nn_Attention (B=4, S=2048, D=2048, H=16, KV=4, HD=128).

Sharding (8 cores): data-parallel over batch (4) x tensor-parallel over
KV-head-group halves (2). Core c handles batch b=c//2 and q-heads
[8*(c%2), 8*(c%2)+8) == kv groups {2*(c%2), 2*(c%2)+1}. Each core produces a
partial output (its heads' contribution through wo); the host sums the two
partials per batch.

Projections (x@wq, x@wkv) run in bf16 in a single merged sweep over x
(shared stationary x tiles, one x DMA; weights stream in behind the first
x tile). RoPE on DVE, then per-head PE transposes produce qT/kT in bf16;
V is stored bf16. Attention is computed transposed (scoresT[k,q]) in bf16
with fp32 PSUM accumulation: per (group) all 4 r's score blocks+exp issue
first (building ACT backlog), then the AV matmuls. Softmax denominators
come from tiny probs-stationary matmuls (out [128q,1] per 128-q-chunk,
~free on PE since LD_WEIGHTS shadows the 1-column stream), then DVE
reciprocal -> four skinny [128,1]->[1,128] PE transposes -> one rank-1
broadcast matmul back to [128,512], all time-sharing one PSUM bank
(matmul start=True zeroes the whole bank, so only each tile's first write
sets it); normalization is fused into the PSUM->SBUF move of the AV
result (DVE multiply). wo stays resident in SBUF (bf16) and the output
projection of superblock qsb-1 is interleaved into the AV phase of qsb so
PE keeps streaming while ACT drains the exp backlog; qsb0 issues both
groups' scores up front (A-A-B-B) since it has no out-proj filler.
The causal mask is folded into the scores matmul via an identity-
stationary accumulation (sc += I @ msk, mask clamped to finite bf16) so
exp never waits on DVE. TimelineSim: ~456us/core vs the 584us baseline;
rel err ~4.7e-3.
"""
import numpy as np

B, S, D = 4, 2048, 2048
H, KV, HD = 16, 4, 128
NREP = H // KV
SCALE = float(HD) ** -0.5

SB = S // 128          # 16 s-blocks
KT = D // 128          # 16 contraction tiles for projections
QSB = S // 512         # 4 q-superblocks
HPC = 8                # q heads per core
GPC = 2                # kv groups per core

_compiled = {}


def _build(causal: bool):
    import concourse.bass as bass  # noqa: F401
    import concourse.tile as tile
    from concourse import bacc, mybir
    from concourse.masks import make_identity

    f32 = mybir.dt.float32
    f32r = mybir.dt.float32r
    bf16 = mybir.dt.bfloat16
    AF = mybir.ActivationFunctionType
    ALU = mybir.AluOpType

    nc = bacc.Bacc("TRN2")

    xT = nc.dram_tensor("xT", [D, S], bf16, kind="ExternalInput")
    wqT = nc.dram_tensor("wqT", [D, HPC * HD], bf16, kind="ExternalInput")
    wkvT = nc.dram_tensor("wkvT", [D, 2 * GPC * HD], bf16, kind="ExternalInput")
    woT = nc.dram_tensor("woT", [HPC * HD, D], bf16, kind="ExternalInput")
    cosS = nc.dram_tensor("cosS", [128, SB, 64], f32, kind="ExternalInput")
    sinS = nc.dram_tensor("sinS", [128, SB, 64], f32, kind="ExternalInput")
    mtile = nc.dram_tensor("mtile", [128, 128], bf16, kind="ExternalInput")
    onest = nc.dram_tensor("onest", [128, 128], bf16, kind="ExternalInput")
    outT = nc.dram_tensor("outT", [D, S], bf16, kind="ExternalOutput")

    xT3 = xT.rearrange("(kt p) s -> p kt s", p=128)
    wqT3 = wqT.rearrange("(kt p) e -> p kt e", p=128)
    wkvT3 = wkvT.rearrange("(kt p) e -> p kt e", p=128)
    woT3 = woT.rearrange("(h p) d -> p h d", p=128)

    with tile.TileContext(nc) as tc:
        with tc.tile_pool(name="persist", bufs=1) as persist:
            qT = [persist.tile([128, S], bf16, tag=f"qT{h}", name=f"qT{h}") for h in range(HPC)]
            kT = [persist.tile([128, S], bf16, tag=f"kTg{g}", name=f"kTg{g}") for g in range(GPC)]
            vsb = [persist.tile([128, SB, 128], bf16, tag=f"v{g}", name=f"v{g}") for g in range(GPC)]
            msk = persist.tile([128, 128], bf16, tag="msk")
            ones = persist.tile([128, 128], bf16, tag="ones")
            identp = persist.tile([128, 128], bf16, tag="identp")

            # ---------- Stage 1: merged q+kv projections + RoPE ----------
            s1ctx = tc.tile_pool(name="s1const", bufs=1)
            s1const = s1ctx.__enter__()
            ident_f = s1const.tile([128, 128], f32, tag="identf")
            make_identity(nc, ident_f)
            ident = identp
            nc.vector.tensor_copy(out=ident, in_=ident_f)
            cos_t = s1const.tile([128, SB, 64], f32, tag="cos")
            sin_t = s1const.tile([128, SB, 64], f32, tag="sin")
            wq_t = s1const.tile([128, KT, HPC * HD], bf16, tag="wq")
            wkv_t = s1const.tile([128, KT, 2 * GPC * HD], bf16, tag="wkv")

            with tc.tile_pool(name="xs1", bufs=2) as xpool, \
                 tc.tile_pool(name="rs1", bufs=2) as rpool, \
                 tc.tile_pool(name="pq1", bufs=2, space="PSUM") as pqp, \
                 tc.tile_pool(name="pk1", bufs=2, space="PSUM") as pkp, \
                 tc.tile_pool(name="pt1", bufs=2, space="PSUM") as ptp:
                for sb in range(SB):
                    xs = xpool.tile([128, KT, 128], bf16, tag="xs")
                    xeng = nc.sync if sb == 0 else nc.scalar
                    xeng.dma_start(
                        out=xs[:, 0:8, :],
                        in_=xT3[:, 0:8, sb * 128:(sb + 1) * 128])
                    xeng.dma_start(
                        out=xs[:, 8:16, :],
                        in_=xT3[:, 8:16, sb * 128:(sb + 1) * 128])
                    if sb == 0:
                        for kt4 in range(0, KT, 2):
                            nc.sync.dma_start(
                                out=wkv_t[:, kt4:kt4 + 2, :],
                                in_=wkvT3[:, kt4:kt4 + 2, :])
                            nc.sync.dma_start(
                                out=wq_t[:, kt4:kt4 + 2, :],
                                in_=wqT3[:, kt4:kt4 + 2, :])
                            if kt4 == 2:
                                nc.sync.dma_start(out=cos_t, in_=cosS[:, :, :])
                                nc.sync.dma_start(out=sin_t, in_=sinS[:, :, :])
                        # mask/ones aren't needed until attention
                        nc.scalar.dma_start(out=msk, in_=mtile[:, :])
                        nc.scalar.dma_start(out=ones, in_=onest[:, :])
                    ps_q = pqp.tile([128, HPC * HD], f32, tag="psq")
                    ps_k = pkp.tile([128, 2 * GPC * HD], f32, tag="psk")
                    kts = list(range(KT))
                    for i, kt in enumerate(kts):
                        nc.tensor.matmul(
                            ps_k, xs[:, kt, :], wkv_t[:, kt, :],
                            start=(i == 0), stop=(i == KT - 1))
                        for n0 in (0, 512):
                            nc.tensor.matmul(
                                ps_q[:, n0:n0 + 512], xs[:, kt, :],
                                wq_t[:, kt, n0:n0 + 512],
                                start=(i == 0), stop=(i == KT - 1))

                    # V copies (no RoPE) — ACT engine
                    ps_k3 = ps_k.rearrange("p (h d) -> p h d", d=128)
                    for g in range(GPC):
                        nc.scalar.copy(
                            out=vsb[g][:, sb, :], in_=ps_k3[:, GPC + g, :])

                    # RoPE on q (8 heads) and k (2 heads)
                    def rope(ps3, h0, nr, tagsfx, rp=None):
                        if rp is None:
                            rp = rpool.tile([128, HPC, 128], bf16,
                                            tag="rp" + tagsfx, name="rp")
                        ev = ps3[:, h0:h0 + nr, 0:128:2]
                        od = ps3[:, h0:h0 + nr, 1:128:2]
                        cb = cos_t[:, None, sb, :].broadcast_to([128, nr, 64])
                        sn = sin_t[:, None, sb, :].broadcast_to([128, nr, 64])
                        t1 = rpool.tile([128, HPC, 64], f32, tag="t1" + tagsfx)
                        t2 = rpool.tile([128, HPC, 64], f32, tag="t2" + tagsfx)
                        hs = slice(h0, h0 + nr)
                        nc.vector.tensor_tensor(
                            out=t1[:, 0:nr, :], in0=ev, in1=cb, op=ALU.mult)
                        nc.vector.tensor_tensor(
                            out=t2[:, 0:nr, :], in0=od, in1=sn, op=ALU.mult)
                        nc.vector.tensor_tensor(
                            out=rp[:, hs, 0:64], in0=t1[:, 0:nr, :],
                            in1=t2[:, 0:nr, :], op=ALU.subtract)
                        nc.vector.tensor_tensor(
                            out=t1[:, 0:nr, :], in0=ev, in1=sn, op=ALU.mult)
                        nc.vector.tensor_tensor(
                            out=t2[:, 0:nr, :], in0=od, in1=cb, op=ALU.mult)
                        nc.vector.tensor_tensor(
                            out=rp[:, hs, 64:128], in0=t1[:, 0:nr, :],
                            in1=t2[:, 0:nr, :], op=ALU.add)
                        return rp

                    def kt_transp(g, rpk):
                        pt = ptp.tile([128, 128], bf16, tag="pt")
                        nc.tensor.transpose(pt, rpk[:, g, :], ident)
                        nc.scalar.copy(
                            out=kT[g][:, sb * 128:(sb + 1) * 128], in_=pt)

                    def qt_transp(h, rpq):
                        pt = ptp.tile([128, 128], bf16, tag="pt")
                        nc.tensor.transpose(pt, rpq[:, h, :], ident)
                        nc.scalar.copy(
                            out=qT[h][:, sb * 128:(sb + 1) * 128], in_=pt)

                    ps_q3 = ps_q.rearrange("p (h d) -> p h d", d=128)
                    rpk = rope(ps_k3, 0, GPC, "k")
                    if sb < SB - 1:
                        rpq = rope(ps_q3, 0, HPC, "q")
                        for g in range(GPC):
                            kt_transp(g, rpk)
                        for h in range(HPC):
                            qt_transp(h, rpq)
                    else:
                        # last s-block: halve the rope->transpose latency so
                        # stage 2 (queued behind the transposes) starts early
                        for g in range(GPC):
                            kt_transp(g, rpk)
                        rpq = rope(ps_q3, 0, 4, "q")
                        for h in range(4):
                            qt_transp(h, rpq)
                        rpq = rope(ps_q3, 4, 4, "q", rp=rpq)
                        for h in range(4, HPC):
                            qt_transp(h, rpq)
            s1ctx.__exit__(None, None, None)

            # ---------- Stage 2: attention + interleaved out-projection ----
            with tc.tile_pool(name="s2", bufs=1) as s2pool, \
                 tc.tile_pool(name="pr2", bufs=5) as prpool, \
                 tc.tile_pool(name="att2", bufs=2) as attpool, \
                 tc.tile_pool(name="dn2", bufs=2) as dnpool, \
                 tc.tile_pool(name="o2", bufs=3) as opool, \
                 tc.tile_pool(name="pav", bufs=2, space="PSUM") as pavp, \
                 tc.tile_pool(name="pdn", bufs=1, space="PSUM") as pdnp, \
                 tc.tile_pool(name="pou", bufs=2, space="PSUM") as poup, \
                 tc.tile_pool(name="psc", bufs=3, space="PSUM") as pscp:
                wos = s2pool.tile([128, HPC, D], bf16, tag="wos")
                for h2 in range(0, HPC, 2):
                    nc.scalar.dma_start(
                        out=wos[:, h2:h2 + 2, :], in_=woT3[:, h2:h2 + 2, :])

                att_prev = None

                def op_block(m, qsb_prev, att_p, split=False):
                    od = outT[m * 128:(m + 1) * 128,
                              qsb_prev * 512:(qsb_prev + 1) * 512]
                    if split:
                        # final block: half-width accumulations so the first
                        # half's copy+DMA drain under the second's matmuls
                        for q0 in (0, 256):
                            poh = poup.tile([128, 256], f32, tag="po", name="poh")
                            for e in range(HPC):
                                nc.tensor.matmul(
                                    poh, wos[:, e, m * 128:(m + 1) * 128],
                                    att_p[:, e, q0:q0 + 256],
                                    start=(e == 0), stop=(e == HPC - 1))
                            oth = opool.tile([128, 256], bf16, tag="ot",
                                             name="oth")
                            nc.vector.tensor_copy(out=oth, in_=poh)
                            nc.sync.dma_start(out=od[:, q0:q0 + 256], in_=oth)
                        return
                    po = poup.tile([128, 512], f32, tag="po")
                    for e in range(HPC):
                        nc.tensor.matmul(
                            po, wos[:, e, m * 128:(m + 1) * 128],
                            att_p[:, e, :], start=(e == 0), stop=(e == HPC - 1))
                    ot = opool.tile([128, 512], bf16, tag="ot")
                    if False:
                        # drain the final block via both copy engines
                        nc.vector.tensor_copy(out=ot[:, 0:256], in_=po[:, 0:256])
                        nc.sync.dma_start(out=od[:, 0:256], in_=ot[:, 0:256])
                        nc.scalar.copy(out=ot[:, 256:512], in_=po[:, 256:512])
                        nc.scalar.dma_start(out=od[:, 256:512],
                                            in_=ot[:, 256:512])
                    else:
                        nc.vector.tensor_copy(out=ot, in_=po)
                        nc.sync.dma_start(out=od, in_=ot)

                def phase_a(qsb, g, maxkt, q0g, tag):
                    prb = []
                    for r in range(NREP):
                        h = g * NREP + r
                        shp = [128, maxkt, 512] if tag == "probs0" \
                            else [128, SB, 512]
                        probs = prpool.tile(shp, bf16, tag=tag)
                        prb.append(probs)
                        for t in range(maxkt):
                            ql = max(0, t * 128 - q0g) if causal else 0
                            sc = pscp.tile([128, 512], f32, tag="sc")
                            nc.tensor.matmul(
                                sc[:, ql:512],
                                kT[g][:, t * 128:(t + 1) * 128],
                                qT[h][:, q0g + ql:q0g + 512],
                                start=True,
                                stop=not (causal and t * 128 >= q0g),
                                skip_group_check=True)
                            is_diag = causal and t * 128 >= q0g
                            if is_diag:
                                # diag block: fold the 0/-inf mask in via an
                                # identity-stationary matmul accumulation
                                # (sc += I @ msk) so exp never waits on DVE;
                                # exp(SCALE*(sc+msk)) underflows to 0
                                # identically for the 0/-inf mask
                                nc.tensor.matmul(
                                    sc[:, ql:ql + 128], identp, msk,
                                    start=False, stop=True,
                                    skip_group_check=True)
                            nc.scalar.activation(
                                out=probs[:, t, ql:512],
                                in_=sc[:, ql:512], func=AF.Exp,
                                scale=SCALE)
                    return prb

                def phase_b(qsb, g, maxkt, q0g, prb, att, it, att_p,
                            pending):
                    # pending: deferred chain-tail (transposes/broadcast/
                    # normalize) of the previous r, emitted one iteration
                    # late so its DVE round-trips hide under AV/out-proj work
                    def chain_tail(av, rrq, h):
                        def emit():
                            # reorient: 4 skinny transposes land each chunk's
                            # recip row at partition 0 of one PSUM row
                            ptc = pdnp.tile([1, 512], bf16, tag="dn")
                            for c in range(4):
                                nc.tensor.matmul(
                                    ptc[0:1, c * 128:(c + 1) * 128],
                                    rrq[:, c:c + 1], identp,
                                    is_transpose=True,
                                    start=(c == 0), stop=(c == 3),
                                    skip_group_check=True)
                            rr = dnpool.tile([1, 512], bf16, tag="rr")
                            nc.vector.tensor_copy(out=rr, in_=ptc)
                            # broadcast recip row to [128, 512]
                            rsb = pdnp.tile([128, 512], f32, tag="dn")
                            nc.tensor.matmul(
                                rsb, ones[0:1, :], rr,
                                start=True, stop=True,
                                skip_group_check=True)
                            rsbs = dnpool.tile([128, 512], bf16, tag="rsbs")
                            nc.vector.tensor_copy(out=rsbs, in_=rsb)
                            # normalize: att = av * rsb (PSUM->SBUF fused)
                            nc.vector.tensor_tensor(
                                out=att[:, h, :], in0=av, in1=rsbs,
                                op=ALU.mult)
                        return emit

                    for r in range(NREP):
                        h = g * NREP + r
                        probs = prb[r]
                        av = pavp.tile([128, 512], f32, tag="av")
                        for t in range(maxkt):
                            ql = max(0, t * 128 - q0g) if causal else 0
                            nc.tensor.matmul(
                                av[:, ql:512], vsb[g][:, t, :],
                                probs[:, t, ql:512],
                                start=(t == 0), stop=(t == maxkt - 1),
                                skip_group_check=True)
                        # denominators: probs-stationary tiny matmuls,
                        # accumulated per 128-q-chunk in PSUM
                        # NOTE: matmul start=True zeroes the whole PSUM
                        # bank, so only the very first write may set it
                        den = pdnp.tile([128, 4], f32, tag="dn")
                        for t in range(maxkt):
                            ql = max(0, t * 128 - q0g) if causal else 0
                            for c in range(ql // 128, 4):
                                tstop = (4 * qsb + c) if causal else (maxkt - 1)
                                nc.tensor.matmul(
                                    den[:, c:c + 1],
                                    probs[:, t, c * 128:(c + 1) * 128],
                                    ones[:, 0:1],
                                    start=(t == 0 and c == 0),
                                    stop=(t == tstop),
                                    skip_group_check=True)
                        rrq = dnpool.tile([128, 4], bf16, tag="rrq")
                        with nc.allow_low_precision(reason="softmax recip"):
                            nc.vector.reciprocal(out=rrq, in_=den)
                        # out-projection fill for the previous superblock
                        # (also gives the reciprocal time to land before the
                        # chain tail needs it)
                        if att_p is not None:
                            for m in (2 * it, 2 * it + 1):
                                op_block(m, qsb - 1, att_p)
                        chain_tail(av, rrq, h)()
                        it += 1
                    return it, pending

                pending = None
                for qsb in range(QSB):
                    att = attpool.tile([128, HPC, 512], bf16, tag="att")
                    maxkt = (qsb + 1) * 4 if causal else SB
                    q0g = qsb * 512
                    it = 0  # (g, r) iteration counter within this qsb
                    if qsb == 0 and causal:
                        # A A B B: both groups' scores issue up front so the
                        # exp backlog drains while PE streams the second
                        # group's scores (no out-proj filler exists yet)
                        prbs = [phase_a(qsb, g, maxkt, q0g,
                                        "probs" if g == 0 else "probs0")
                                for g in range(GPC)]
                        for g in range(GPC):
                            it, pending = phase_b(qsb, g, maxkt, q0g,
                                                  prbs[g], att, it,
                                                  att_prev, pending)
                    else:
                        for g in range(GPC):
                            prb = phase_a(qsb, g, maxkt, q0g, "probs")
                            it, pending = phase_b(qsb, g, maxkt, q0g, prb,
                                                  att, it, att_prev, pending)
                    att_prev = att
                if pending is not None:
                    pending()
                # tail: out-projection of the last superblock
                for m in range(KT):
                    op_block(m, QSB - 1, att_prev, split=(m == KT - 1))

    nc.compile()
    return nc


def _get_nc(causal: bool):
    if causal not in _compiled:
        _compiled[causal] = _build(causal)
    return _compiled[causal]


def kernel(x, freqs_cis, mask, wq, wk, wv, wo):
    import ml_dtypes
    from concourse.bass_utils import run_bass_kernel_spmd

    x = np.asarray(x, dtype=np.float32)
    freqs_cis = np.asarray(freqs_cis, dtype=np.float32)
    mask = np.asarray(mask, dtype=np.float32)
    wq = np.asarray(wq, dtype=np.float32)
    wk = np.asarray(wk, dtype=np.float32)
    wv = np.asarray(wv, dtype=np.float32)
    wo = np.asarray(wo, dtype=np.float32)

    tri = np.tril(np.ones((S, S), dtype=bool))
    causal = bool((mask[tri] == 0.0).all() and (mask[~tri] < -1e30).all())
    if not causal and not (mask == 0.0).all():
        return _numpy_ref(x, freqs_cis, mask, wq, wk, wv, wo)

    nc = _get_nc(causal)

    cos = freqs_cis[:, :, 0]
    sin = freqs_cis[:, :, 1]
    cosS = np.ascontiguousarray(cos.reshape(SB, 128, 64).transpose(1, 0, 2))
    sinS = np.ascontiguousarray(sin.reshape(SB, 128, 64).transpose(1, 0, 2))
    # clamp to a finite bf16: -float32-max would round to -inf, and the
    # identity-matmul mask fold would then compute 0 * -inf = NaN
    mtile = np.maximum(
        (np.ascontiguousarray(mask[0:128, 0:128].T) if causal
         else np.zeros((128, 128), dtype=np.float32)), -3.0e38
    ).astype(ml_dtypes.bfloat16)
    onest = np.ones((128, 128), dtype=ml_dtypes.bfloat16)

    in_maps = []
    for c in range(8):
        b, i = c // 2, c % 2
        in_maps.append({
            "xT": np.ascontiguousarray(x[b].T).astype(ml_dtypes.bfloat16),
            "wqT": np.ascontiguousarray(
                wq[1024 * i:1024 * (i + 1), :].T).astype(ml_dtypes.bfloat16),
            "wkvT": np.ascontiguousarray(np.concatenate(
                [wk[256 * i:256 * (i + 1), :].T,
                 wv[256 * i:256 * (i + 1), :].T],
                axis=1)).astype(ml_dtypes.bfloat16),
            "woT": np.ascontiguousarray(
                wo[:, 1024 * i:1024 * (i + 1)].T).astype(ml_dtypes.bfloat16),
            "cosS": cosS, "sinS": sinS, "mtile": mtile, "onest": onest,
        })

    out = None
    for _attempt in range(2):
        res = run_bass_kernel_spmd(nc, in_maps, core_ids=list(range(8)))
        out = np.empty((B, S, D), dtype=np.float32)
        for b in range(B):
            o0 = res.results[2 * b]["outT"].astype(np.float32)
            o1 = res.results[2 * b + 1]["outT"].astype(np.float32)
            out[b] = o0.T + o1.T
        if np.isfinite(out).all():
            break
        # transient device-warmup glitch observed once on the axon path:
        # retry a single time rather than returning garbage
    return out


def _numpy_ref(x, freqs_cis, mask, wq, wk, wv, wo):
    xq = (x @ wq.T).reshape(B, S, H, HD)
    xk = (x @ wk.T).reshape(B, S, KV, HD)
    xv = (x @ wv.T).reshape(B, S, KV, HD)

    def rope(xh):
        x2 = xh.reshape(*xh.shape[:-1], HD // 2, 2)
        fc = freqs_cis[None, :, None, :, :]
        real = x2[..., 0] * fc[..., 0] - x2[..., 1] * fc[..., 1]
        imag = x2[..., 0] * fc[..., 1] + x2[..., 1] * fc[..., 0]
        return np.concatenate([real, imag], axis=-1)

    xq, xk = rope(xq), rope(xk)
    q = xq.reshape(B, S, KV, NREP, HD)
    sc = np.einsum('bqgrd,bkgd->bgrqk', q, xk) * SCALE + mask[None, None, None]
    sc = sc - sc.max(axis=-1, keepdims=True)
    p = np.exp(sc)
    p /= p.sum(axis=-1, keepdims=True)
    o = np.einsum('bgrqk,bkgd->bqgrd', p, xv).reshape(B, S, H * HD)
    return (o @ wo.T).astype(np.float32)
